# revision 13
# baseline (speedup 1.0000x reference)
"""Trainium2 Bass kernel for DiagonalLinear: y = x * diag (elementwise scale of last axis).

x: [4, 4096, 4096] f32, diag: [4096] f32 -> y: [4, 4096, 4096] f32.

Data-parallel over 8 NeuronCores: the 16384 rows (batch*seq) are split into
8 equal shards of 2048 rows; diag is replicated. Each core streams its
32 MiB shard through SBUF in 16 tiles of [128 partitions x 4096 floats]
(one row per partition), multiplies by a diag tile replicated across
partitions (diag is loaded once, 16 KiB, and broadcast on-chip via a
TensorE outer product), and streams the result back out. Memory-bound:
~64 MiB of HBM traffic per core; ~175 us/core at the ~420 GB/s fabric
ceiling, up to ~210 us on cores throttled to their HBM fair share.
"""

import numpy as np

import concourse.bass as bass
import concourse.bacc as bacc
import concourse.mybir as mybir
import concourse.tile as tile
from concourse.bass_utils import run_bass_kernel_spmd

BATCH, SEQ, SIZE = 4, 4096, 4096
N_CORES = 8
ROWS = BATCH * SEQ                   # 16384
ROWS_PER_CORE = ROWS // N_CORES      # 2048
P = 128                              # SBUF partitions
F = 4096                             # free-dim elements per partition per tile
ROWS_PER_PART = F // SIZE            # rows per partition (1)
T = ROWS_PER_CORE * SIZE // (P * F)  # 16 tiles of 2 MiB per core
FP32 = mybir.dt.float32

_built = None


def _build():
    """Build + schedule the per-core Tile kernel (same program on all 8 cores)."""
    nc = bacc.Bacc("TRN2", target_bir_lowering=False, debug=False)

    x = nc.dram_tensor("x", [T, P, F], FP32, kind="ExternalInput").ap()
    d = nc.dram_tensor("diag", [SIZE], FP32, kind="ExternalInput").ap()
    y = nc.dram_tensor("y", [T, P, F], FP32, kind="ExternalOutput").ap()

    with tile.TileContext(nc) as tc:
        with (
            tc.tile_pool(name="dpool", bufs=1) as dpool,
            tc.tile_pool(name="ppool", bufs=1, space="PSUM") as ppool,
            tc.tile_pool(name="xpool", bufs=9) as xpool,
        ):
            # Load diag once (16 KiB) into partition 0, then replicate it
            # across all 128 partitions via a TensorE outer product
            # ones[128,1] @ diag[1,SIZE] — PE and PSUM are otherwise idle,
            # and this avoids both bulk HBM traffic and the GpSimd custom-op
            # library load that partition_broadcast would pull in.
            d0 = dpool.tile([1, SIZE], FP32)
            nc.sync.dma_start(out=d0[:], in_=d[None, :])
            ones = dpool.tile([1, P], FP32)
            nc.vector.memset(ones[:], 1.0)
            ptile = ppool.tile([P, SIZE], FP32)
            for j in range(SIZE // 512):
                nc.tensor.matmul(
                    ptile[:, j * 512 : (j + 1) * 512],
                    ones[:],
                    d0[:, j * 512 : (j + 1) * 512],
                    start=True,
                    stop=True,
                )
            dtile = dpool.tile([P, SIZE], FP32)
            nc.vector.tensor_copy(dtile[:], ptile[:])

            # Alternate which HWDGE ring (SP vs ACT) carries each tile's
            # load/store: halves the per-ring serial dispatch cost and keeps
            # both rings' queues deep so SDMA engines always see ready work.
            for t in range(T):
                xt = xpool.tile([P, F], FP32)
                ld_eng, st_eng = (
                    (nc.sync, nc.scalar) if t % 2 == 0 else (nc.scalar, nc.sync)
                )
                ld_eng.dma_start(out=xt[:], in_=x[t])
                for j in range(ROWS_PER_PART):
                    sl = xt[:, j * SIZE : (j + 1) * SIZE]
                    nc.vector.tensor_mul(sl, sl, dtile[:])
                st_eng.dma_start(out=y[t], in_=xt[:])

    nc.compile()
    return nc


def _get_nc():
    global _built
    if _built is None:
        _built = _build()
    return _built


def _make_in_maps(x: np.ndarray, diag: np.ndarray):
    xs = np.ascontiguousarray(np.asarray(x, dtype=np.float32)).reshape(
        N_CORES, T, P, F
    )
    dg = np.ascontiguousarray(np.asarray(diag, dtype=np.float32))
    return [{"x": xs[i], "diag": dg} for i in range(N_CORES)]


def _assemble(results) -> np.ndarray:
    out = np.stack([results[i]["y"] for i in range(N_CORES)])
    return out.reshape(BATCH, SEQ, SIZE)


def kernel(x: np.ndarray, diag: np.ndarray) -> np.ndarray:
    nc = _get_nc()
    res = run_bass_kernel_spmd(nc, _make_in_maps(x, diag), list(range(N_CORES)))
    return _assemble(res.results)


# revision 15
# speedup vs baseline: 1.0439x; 1.0439x over previous
"""Trainium2 Bass kernel for DiagonalLinear: y = x * diag (elementwise scale of last axis).

x: [4, 4096, 4096] f32, diag: [4096] f32 -> y: [4, 4096, 4096] f32.

Data-parallel over 8 NeuronCores: the 16384 rows (batch*seq) are split into
8 equal shards of 2048 rows; diag is replicated. Each core streams its
32 MiB shard through SBUF in 16 tiles of [128 partitions x 4096 floats]
(one row per partition), multiplies by a diag tile replicated across
partitions (diag is loaded once, 16 KiB, and broadcast on-chip via a
TensorE outer product), and streams the result back out. Memory-bound:
~64 MiB of HBM traffic per core; ~175 us/core at the ~420 GB/s fabric
ceiling, up to ~210 us on cores throttled to their HBM fair share.
"""

import numpy as np

import concourse.bass as bass
import concourse.bacc as bacc
import concourse.mybir as mybir
import concourse.tile as tile
from concourse.bass_utils import run_bass_kernel_spmd

BATCH, SEQ, SIZE = 4, 4096, 4096
N_CORES = 8
ROWS = BATCH * SEQ                   # 16384
ROWS_PER_CORE = ROWS // N_CORES      # 2048
P = 128                              # SBUF partitions
F = 4096                             # free-dim elements per partition per tile
ROWS_PER_PART = F // SIZE            # rows per partition (1)
T = ROWS_PER_CORE * SIZE // (P * F)  # 16 tiles of 2 MiB per core
FP32 = mybir.dt.float32

_built = None


def _build():
    """Build + schedule the per-core Tile kernel (same program on all 8 cores)."""
    nc = bacc.Bacc("TRN2", target_bir_lowering=False, debug=False)

    x = nc.dram_tensor("x", [T, P, F], FP32, kind="ExternalInput").ap()
    d = nc.dram_tensor("diag", [SIZE], FP32, kind="ExternalInput").ap()
    y = nc.dram_tensor("y", [T, P, F], FP32, kind="ExternalOutput").ap()

    with tile.TileContext(nc) as tc:
        with (
            tc.tile_pool(name="dpool", bufs=1) as dpool,
            tc.tile_pool(name="ppool", bufs=1, space="PSUM") as ppool,
            tc.tile_pool(name="xpool", bufs=8) as xpool,
        ):
            # Load diag once (16 KiB) into partition 0, then replicate it
            # across all 128 partitions via a TensorE outer product
            # ones[128,1] @ diag[1,SIZE] — PE and PSUM are otherwise idle,
            # and this avoids both bulk HBM traffic and the GpSimd custom-op
            # library load that partition_broadcast would pull in.
            d0 = dpool.tile([1, SIZE], FP32)
            nc.sync.dma_start(out=d0[:], in_=d[None, :])
            ones = dpool.tile([1, P], FP32)
            nc.vector.memset(ones[:], 1.0)
            ptile = ppool.tile([P, SIZE], FP32)
            for j in range(SIZE // 512):
                nc.tensor.matmul(
                    ptile[:, j * 512 : (j + 1) * 512],
                    ones[:],
                    d0[:, j * 512 : (j + 1) * 512],
                    start=True,
                    stop=True,
                )
            dtile = dpool.tile([P, SIZE], FP32)
            nc.vector.tensor_copy(dtile[:], ptile[:])

            # Loads ride the SP HWDGE ring, stores the ACT ring — two
            # independent FIFO streams feeding the shared 16 SDMA engines.
            for t in range(T):
                xt = xpool.tile([P, F], FP32)
                nc.sync.dma_start(out=xt[:], in_=x[t])
                for j in range(ROWS_PER_PART):
                    sl = xt[:, j * SIZE : (j + 1) * SIZE]
                    nc.vector.tensor_mul(sl, sl, dtile[:])
                nc.scalar.dma_start(out=y[t], in_=xt[:])

    nc.compile()
    return nc


def _get_nc():
    global _built
    if _built is None:
        _built = _build()
    return _built


def _make_in_maps(x: np.ndarray, diag: np.ndarray):
    xs = np.ascontiguousarray(np.asarray(x, dtype=np.float32)).reshape(
        N_CORES, T, P, F
    )
    dg = np.ascontiguousarray(np.asarray(diag, dtype=np.float32))
    return [{"x": xs[i], "diag": dg} for i in range(N_CORES)]


def _assemble(results) -> np.ndarray:
    out = np.stack([results[i]["y"] for i in range(N_CORES)])
    return out.reshape(BATCH, SEQ, SIZE)


def kernel(x: np.ndarray, diag: np.ndarray) -> np.ndarray:
    nc = _get_nc()
    res = run_bass_kernel_spmd(nc, _make_in_maps(x, diag), list(range(N_CORES)))
    return _assemble(res.results)
